# Initial kernel scaffold
#
"""Depth2Normal Trainium2 kernel.

Computes, per batch image: camera-space points from depth + intrinsics, and
per-pixel surface normals via a 9x9-window least-squares plane fit
(AtA n = Atb solved with Cramer's rule; singular windows fall back to Atb),
matching reference.py semantics.

Sharding: pure data parallel, one batch element per NeuronCore (B=8 = 8 cores).

Per-core layout: partitions = H rows (4 tiles of 120 output rows + 4 halo on
each side = 128), free dim = W (640 + 4 zero-pad each side = 648).

Box sums: both window directions run on the TensorEngine. For each output
W-chunk, 9 accumulating matmuls (one per W-shift s in 0..8) with a banded
[128,120] ones matrix as the stationary operand compute the full 2D 9x9 sum
into PSUM. ScalarE evacuates PSUM->SBUF; the per-pixel 3x3 solve runs on the
VectorEngine.
"""

import sys

sys.path.insert(0, "/opt/trn_rl_repo")

import numpy as np

from concourse import bass, mybir, tile
from concourse.bass_utils import run_bass_kernel_spmd

F32 = mybir.dt.float32
AF = mybir.ActivationFunctionType
OP = mybir.AluOpType

B, H, W = 8, 480, 640
K_SIZE = 9
HALO = K_SIZE // 2  # 4
TILE_H = 120  # output rows per H-tile
N_TILES = H // TILE_H  # 4
PW = W + 2 * HALO  # padded width 648
CW = 320  # W-chunk width (PSUM bank holds 512 f32; 320*4B=1280B)
N_CHUNKS = W // CW  # 2
DEPTH_MAX = 10.0
DET_EPS = 1e-5
NORM_EPS = 1e-5


def _build_program(n_cores=8):
    nc = bass.Bass(
        "TRN2",
        target_bir_lowering=False,
        debug=False,
        enable_asserts=False,
        num_devices=n_cores,
    )
    d_depth = nc.dram_tensor("depth", [H, W], F32, kind="ExternalInput").ap()
    d_band = nc.dram_tensor("band", [128, TILE_H], F32, kind="ExternalInput").ap()
    d_xg = nc.dram_tensor("xg", [1, PW], F32, kind="ExternalInput").ap()
    d_ysc = nc.dram_tensor("ysc", [128, 3 * N_TILES], F32, kind="ExternalInput").ap()
    d_kx = nc.dram_tensor("kx", [128, 3], F32, kind="ExternalInput").ap()
    d_norm = nc.dram_tensor("normals", [3, H, W], F32, kind="ExternalOutput").ap()
    d_pts = nc.dram_tensor("points", [3, H, W], F32, kind="ExternalOutput").ap()

    with tile.TileContext(nc) as tc:
        _body(tc, d_depth, d_band, d_xg, d_ysc, d_kx, d_norm, d_pts)
    return nc


def _body(tc, d_depth, d_band, d_xg, d_ysc, d_kx, d_norm, d_pts):
    nc = tc.nc
    from contextlib import ExitStack

    with ExitStack() as ctx:
        const = ctx.enter_context(tc.tile_pool(name="const", bufs=1))
        io = ctx.enter_context(tc.tile_pool(name="io", bufs=2))
        chp = ctx.enter_context(tc.tile_pool(name="ch", bufs=2))
        ps = ctx.enter_context(tc.tile_pool(name="ps", bufs=8, space="PSUM"))
        sv = ctx.enter_context(tc.tile_pool(name="sv", bufs=2))

        # --- constants ---
        band = const.tile([128, TILE_H], F32)
        nc.sync.dma_start(band[:], d_band[:])
        ysc = const.tile([128, 3 * N_TILES], F32)
        nc.sync.dma_start(ysc[:], d_ysc[:])
        kx = const.tile([128, 3], F32)
        nc.sync.dma_start(kx[:], d_kx[:])
        xrow = const.tile([1, PW], F32)
        nc.sync.dma_start(xrow[:], d_xg[:])
        xgb = const.tile([128, PW], F32)
        nc.gpsimd.partition_broadcast(xgb[:], xrow[:])

        for t in range(N_TILES):
            # --- load depth tile (128 rows: TILE_H + halo), zero-padded ---
            dt_ = io.tile([128, PW], F32, tag="depth")
            nc.gpsimd.memset(dt_[:], 0.0)
            rows0 = t * TILE_H - HALO
            r_lo = max(rows0, 0)
            r_hi = min(rows0 + 128, H)
            p_lo = r_lo - rows0
            nrows = r_hi - r_lo
            nc.sync.dma_start(
                dt_[p_lo : p_lo + nrows, HALO : HALO + W], d_depth[r_lo:r_hi, :]
            )

            # --- dmask = depth * (depth>0) * (depth<DEPTH_MAX) ---
            v1 = io.tile([128, PW], F32, tag="v1")
            nc.vector.tensor_scalar(v1[:], dt_[:], DEPTH_MAX, None, op0=OP.is_lt)
            v2 = io.tile([128, PW], F32, tag="v2")
            nc.vector.scalar_tensor_tensor(
                v2[:], dt_[:], 0.0, v1[:], op0=OP.is_gt, op1=OP.mult
            )
            dmask = io.tile([128, PW], F32, tag="dmask")
            nc.vector.tensor_mul(dmask[:], dt_[:], v2[:])

            # --- P_i, cam (points out), mp (masked points) ---
            mp = []
            for i in range(3):
                Pi = io.tile([128, PW], F32, tag=f"P{i}")
                nc.scalar.activation(
                    Pi[:],
                    xgb[:],
                    AF.Identity,
                    bias=ysc[:, 3 * t + i : 3 * t + i + 1],
                    scale=kx[:, i : i + 1],
                )
                cam_i = io.tile([128, PW], F32, tag=f"cam{i}")
                nc.vector.tensor_mul(cam_i[:], Pi[:], dt_[:])
                nc.sync.dma_start(
                    d_pts[i, t * TILE_H : (t + 1) * TILE_H, :],
                    cam_i[HALO : HALO + TILE_H, HALO : HALO + W],
                )
                mp_i = chp.tile([128, PW], F32, tag=f"mp{i}")
                nc.vector.tensor_mul(mp_i[:], Pi[:], dmask[:])
                mp.append(mp_i)

            # --- outer-product channels (6 unique) ---
            # order: s00 s01 s02 s11 s12 s22 b0 b1 b2
            chans = []
            for ci, (i, j) in enumerate([(0, 0), (0, 1), (0, 2), (1, 1), (1, 2), (2, 2)]):
                o = chp.tile([128, PW], F32, tag=f"o{ci}")
                if i == j:
                    nc.scalar.square(o[:], mp[i][:])
                else:
                    nc.vector.tensor_mul(o[:], mp[i][:], mp[j][:])
                chans.append(o)
            chans += mp

            for c in range(N_CHUNKS):
                j0 = c * CW
                # --- 2D box sums on TensorE: 9 shifted accumulating matmuls ---
                evs = []
                for ci, chan in enumerate(chans):
                    pt = ps.tile([TILE_H, CW], F32, tag="ps")
                    for s in range(K_SIZE):
                        nc.tensor.matmul(
                            pt[:],
                            band[:],
                            chan[:, j0 + s : j0 + s + CW],
                            start=(s == 0),
                            stop=(s == K_SIZE - 1),
                        )
                    ev = sv.tile([TILE_H, CW], F32, tag=f"ev{ci}")
                    nc.scalar.copy(ev[:], pt[:])
                    evs.append(ev)
                s00, s01, s02, s11, s12, s22, b0, b1, b2 = evs

                def tmp(tag):
                    return sv.tile([TILE_H, CW], F32, tag=tag)

                mul = nc.vector.tensor_mul
                sub = nc.vector.tensor_sub
                add = nc.vector.tensor_add

                # cofactor/Cramer terms
                sq12 = tmp("sq12")
                nc.scalar.square(sq12[:], s12[:])
                ta = tmp("ta")
                tb = tmp("tb")
                M1 = tmp("M1")
                mul(ta[:], s11[:], s22[:])
                sub(M1[:], ta[:], sq12[:])
                M2 = tmp("M2")
                mul(ta[:], b1[:], s22[:])
                mul(tb[:], s12[:], b2[:])
                sub(M2[:], ta[:], tb[:])
                M3 = tmp("M3")
                mul(ta[:], b1[:], s12[:])
                mul(tb[:], s11[:], b2[:])
                sub(M3[:], ta[:], tb[:])
                M4 = tmp("M4")
                mul(ta[:], s01[:], s22[:])
                mul(tb[:], s02[:], s12[:])
                sub(M4[:], ta[:], tb[:])
                M5 = tmp("M5")
                mul(ta[:], s01[:], b2[:])
                mul(tb[:], s02[:], b1[:])
                sub(M5[:], ta[:], tb[:])
                M6 = tmp("M6")
                mul(ta[:], s01[:], s12[:])
                mul(tb[:], s02[:], s11[:])
                sub(M6[:], ta[:], tb[:])

                # det = s00*M1 - s01*M4 + s02*M6
                det = tmp("det")
                mul(ta[:], s00[:], M1[:])
                mul(tb[:], s01[:], M4[:])
                sub(det[:], ta[:], tb[:])
                mul(ta[:], s02[:], M6[:])
                add(det[:], det[:], ta[:])

                # Cramer numerators
                d0 = tmp("d0")
                mul(ta[:], b0[:], M1[:])
                mul(tb[:], s01[:], M2[:])
                sub(d0[:], ta[:], tb[:])
                mul(ta[:], s02[:], M3[:])
                add(d0[:], d0[:], ta[:])
                d1 = tmp("d1")
                mul(ta[:], s00[:], M2[:])
                mul(tb[:], b0[:], M4[:])
                sub(d1[:], ta[:], tb[:])
                mul(ta[:], s02[:], M5[:])
                add(d1[:], d1[:], ta[:])
                d2 = tmp("d2")
                mul(ta[:], b0[:], M6[:])
                mul(tb[:], s00[:], M3[:])
                sub(d2[:], ta[:], tb[:])
                mul(ta[:], s01[:], M5[:])
                sub(d2[:], d2[:], ta[:])

                # good = det >= DET_EPS (catches NaN too: NaN>=eps is false).
                # Good pixels: n ∝ d_i (1/det > 0 scale dropped — normalize
                # makes it irrelevant up to the +eps in the denominator, which
                # is far below this problem's f32 conditioning noise).
                # Bad pixels: n ∝ Atb (AtA replaced by I).
                good = tmp("good")
                nc.vector.tensor_scalar(good[:], det[:], DET_EPS, None, op0=OP.is_ge)
                m = []
                for mi, bi, di in ((0, b0, d0), (1, b1, d1), (2, b2, d2)):
                    m_i = tmp(f"m{mi}")
                    nc.vector.tensor_copy(m_i[:], bi[:])
                    nc.vector.copy_predicated(m_i[:], good[:], di[:])
                    m.append(m_i)

                # normalize: out_i = m_i / (||m|| + NORM_EPS)
                q = tmp("q")
                mul(ta[:], m[0][:], m[0][:])
                mul(tb[:], m[1][:], m[1][:])
                add(q[:], ta[:], tb[:])
                mul(ta[:], m[2][:], m[2][:])
                add(q[:], q[:], ta[:])
                sn = tmp("sn")
                nc.scalar.sqrt(sn[:], q[:])
                nc.vector.tensor_scalar_add(sn[:], sn[:], NORM_EPS)
                rn = tmp("rn")
                nc.vector.reciprocal_approx_fast(rn[:], sn[:])
                for i in range(3):
                    out_i = tmp(f"out{i}")
                    mul(out_i[:], m[i][:], rn[:])
                    nc.sync.dma_start(
                        d_norm[i, t * TILE_H : (t + 1) * TILE_H, j0 : j0 + CW],
                        out_i[:],
                    )


def _host_constants(Kinv):
    """Per-core constant tensors derived from one [3,3] inverse intrinsic."""
    Ki = Kinv.astype(np.float32)
    band = np.zeros((128, TILE_H), np.float32)
    for k in range(128):
        lo, hi = max(0, k - 2 * HALO), min(TILE_H - 1, k)
        if lo <= hi:
            band[k, lo : hi + 1] = 1.0
    xg = (np.arange(PW, dtype=np.float32) - HALO)[None, :]
    ysc = np.zeros((128, 3 * N_TILES), np.float32)
    p = np.arange(128, dtype=np.float32)
    for t in range(N_TILES):
        hrow = t * TILE_H - HALO + p
        for i in range(3):
            ysc[:, 3 * t + i] = Ki[i, 1] * hrow + Ki[i, 2]
    kx = np.tile(Ki[:, 0][None, :], (128, 1)).astype(np.float32)
    return {"band": band, "xg": xg, "ysc": ysc, "kx": kx}


_PROGRAM = None


def _program():
    global _PROGRAM
    if _PROGRAM is None:
        _PROGRAM = _build_program()
    return _PROGRAM


def _run(depth, intrinsic_inv, trace=False, **kw):
    nc = _program()
    in_maps = []
    for b in range(B):
        m = {"depth": np.ascontiguousarray(depth[b], dtype=np.float32)}
        m.update(_host_constants(np.asarray(intrinsic_inv[b])))
        in_maps.append(m)
    return run_bass_kernel_spmd(nc, in_maps, core_ids=list(range(B)), trace=trace, **kw)


def kernel(depth, intrinsic_inv):
    res = _run(np.asarray(depth), np.asarray(intrinsic_inv), trace=False)
    normals = np.stack([res.results[b]["normals"] for b in range(B)])
    points = np.stack([res.results[b]["points"] for b in range(B)])
    return normals.astype(np.float32), points.astype(np.float32)


# revision 18
# speedup vs baseline: 1.0752x; 1.0752x over previous
"""Depth2Normal Trainium2 kernel.

Computes, per batch image: camera-space points from depth + intrinsics, and
per-pixel surface normals via a 9x9-window least-squares plane fit
(AtA n = Atb solved with Cramer's rule; singular windows fall back to Atb),
matching reference.py semantics.

Sharding: pure data parallel, one batch element per NeuronCore (B=8 = 8 cores).

Per-core layout: partitions = H rows (4 tiles of 120 output rows + 4 halo on
each side = 128), free dim = W (640 + 4 zero-pad each side = 648).

Box sums: both window directions run on the TensorEngine. For each output
W-chunk, 9 accumulating matmuls (one per W-shift s in 0..8) with a banded
[128,120] ones matrix as the stationary operand compute the full 2D 9x9 sum
into PSUM. ScalarE evacuates PSUM->SBUF; the per-pixel 3x3 solve runs on the
VectorEngine.
"""

import sys

sys.path.insert(0, "/opt/trn_rl_repo")

import numpy as np

from concourse import bacc, bass, mybir, tile
from concourse.bass_utils import run_bass_kernel_spmd

F32 = mybir.dt.float32
AF = mybir.ActivationFunctionType
OP = mybir.AluOpType

B, H, W = 8, 480, 640
K_SIZE = 9
HALO = K_SIZE // 2  # 4
TILE_H = 120  # output rows per H-tile
N_TILES = H // TILE_H  # 4
PW = W + 2 * HALO  # padded width 648
CW = 320  # W-chunk width (PSUM bank holds 512 f32; 320*4B=1280B)
N_CHUNKS = W // CW  # 2
DEPTH_MAX = 10.0
DET_EPS = 1e-5
NORM_EPS = 1e-5


def _build_program(n_cores=8):
    nc = bacc.Bacc(
        "TRN2",
        target_bir_lowering=False,
        debug=False,
        enable_asserts=False,
        num_devices=n_cores,
    )
    d_depth = nc.dram_tensor("depth", [H, W], F32, kind="ExternalInput").ap()
    # consts layout: [0:120]=band, [120:132]=ysc, [132:135]=kx, [135:135+PW]=xgrid
    d_const = nc.dram_tensor(
        "consts", [128, TILE_H + 3 * N_TILES + 3 + PW], F32, kind="ExternalInput"
    ).ap()
    d_norm = nc.dram_tensor("normals", [3, H, W], F32, kind="ExternalOutput").ap()
    d_pts = nc.dram_tensor("points", [3, H, W], F32, kind="ExternalOutput").ap()

    with tile.TileContext(nc) as tc:
        _body(tc, d_depth, d_const, d_norm, d_pts)
    nc.compile()
    return nc


def _body(tc, d_depth, d_const, d_norm, d_pts):
    nc = tc.nc
    from contextlib import ExitStack

    with ExitStack() as ctx:
        const = ctx.enter_context(tc.tile_pool(name="const", bufs=1))
        io = ctx.enter_context(tc.tile_pool(name="io", bufs=2))
        chp = ctx.enter_context(tc.tile_pool(name="ch", bufs=2))
        ps = ctx.enter_context(tc.tile_pool(name="ps", bufs=8, space="PSUM"))
        sv = ctx.enter_context(tc.tile_pool(name="sv", bufs=2))

        # --- constants (one DMA; slice views) ---
        NC = TILE_H + 3 * N_TILES + 3 + PW
        ct = const.tile([128, NC], F32)
        nc.sync.dma_start(ct[:], d_const[:])
        band = ct[:, 0:TILE_H]
        ysc = ct[:, TILE_H : TILE_H + 3 * N_TILES]
        kx = ct[:, TILE_H + 3 * N_TILES : TILE_H + 3 * N_TILES + 3]
        xgb = ct[:, TILE_H + 3 * N_TILES + 3 : NC]

        for t in range(N_TILES):
            # --- load depth tile (128 rows: TILE_H + halo), zero-padded ---
            dt_ = io.tile([128, PW], F32, tag="depth")
            nc.gpsimd.memset(dt_[:], 0.0)
            rows0 = t * TILE_H - HALO
            r_lo = max(rows0, 0)
            r_hi = min(rows0 + 128, H)
            p_lo = r_lo - rows0
            nrows = r_hi - r_lo
            nc.sync.dma_start(
                dt_[p_lo : p_lo + nrows, HALO : HALO + W], d_depth[r_lo:r_hi, :]
            )

            # --- dmask = depth * (depth>0) * (depth<DEPTH_MAX) ---
            v1 = io.tile([128, PW], F32, tag="v1")
            nc.vector.tensor_scalar(v1[:], dt_[:], DEPTH_MAX, None, op0=OP.is_lt)
            v2 = io.tile([128, PW], F32, tag="v2")
            nc.vector.scalar_tensor_tensor(
                v2[:], dt_[:], 0.0, v1[:], op0=OP.is_gt, op1=OP.mult
            )
            dmask = io.tile([128, PW], F32, tag="dmask")
            nc.vector.tensor_mul(dmask[:], dt_[:], v2[:])

            # --- P_i, cam (points out), mp (masked points) ---
            mp = []
            for i in range(3):
                Pi = io.tile([128, PW], F32, tag=f"P{i}")
                nc.scalar.activation(
                    Pi[:],
                    xgb,
                    AF.Identity,
                    bias=ysc[:, 3 * t + i : 3 * t + i + 1],
                    scale=kx[:, i : i + 1],
                )
                cam_i = io.tile([128, PW], F32, tag=f"cam{i}")
                nc.vector.tensor_mul(cam_i[:], Pi[:], dt_[:])
                nc.sync.dma_start(
                    d_pts[i, t * TILE_H : (t + 1) * TILE_H, :],
                    cam_i[HALO : HALO + TILE_H, HALO : HALO + W],
                )
                mp_i = chp.tile([128, PW], F32, tag=f"mp{i}")
                nc.vector.tensor_mul(mp_i[:], Pi[:], dmask[:])
                mp.append(mp_i)

            # --- outer-product channels (6 unique) ---
            # order: s00 s01 s02 s11 s12 s22 b0 b1 b2
            chans = []
            for ci, (i, j) in enumerate([(0, 0), (0, 1), (0, 2), (1, 1), (1, 2), (2, 2)]):
                o = chp.tile([128, PW], F32, tag=f"o{ci}")
                if i == j:
                    nc.scalar.square(o[:], mp[i][:])
                else:
                    nc.vector.tensor_mul(o[:], mp[i][:], mp[j][:])
                chans.append(o)
            chans += mp

            for c in range(N_CHUNKS):
                j0 = c * CW
                NW = CW + K_SIZE - 1  # 328: input cols for this chunk's W-window
                # --- box sums: H-band on TensorE, then W-window as a
                # running-difference scan on DVE (state = (P[t] + state),
                # i.e. cumsum into cs[1:]; box[j] = cs[j+9] - cs[j]) ---
                boxes = []
                for ci, chan in enumerate(chans):
                    pt = ps.tile([TILE_H, NW], F32, tag="ps")
                    nc.tensor.matmul(
                        pt[:], band, chan[:, j0 : j0 + NW], start=True, stop=True
                    )
                    cs = sv.tile(
                        [TILE_H, NW + 1], F32, tag=f"cs{ci}", name=f"cs{ci}", bufs=1
                    )
                    nc.vector.memset(cs[:, 0:1], 0.0)
                    nc.vector.tensor_tensor_scan(
                        cs[:, 1 : NW + 1],
                        pt[:],
                        ct[:TILE_H, 0:NW],
                        0.0,
                        OP.add,
                        OP.bypass,
                    )
                    box = sv.tile([TILE_H, CW], F32, tag=f"ev{ci}", name=f"ev{ci}")
                    nc.vector.tensor_sub(
                        box[:], cs[:, K_SIZE : K_SIZE + CW], cs[:, 0:CW]
                    )
                    boxes.append(box)
                s00, s01, s02, s11, s12, s22, b0, b1, b2 = boxes

                def tmp(tag):
                    return sv.tile([TILE_H, CW], F32, tag=tag, name=tag)

                mul = nc.vector.tensor_mul
                sub = nc.vector.tensor_sub
                add = nc.vector.tensor_add
                gmul = lambda o, a, b: nc.gpsimd.tensor_tensor(o, a, b, OP.mult)
                gsub = lambda o, a, b: nc.gpsimd.tensor_tensor(o, a, b, OP.subtract)
                gadd = lambda o, a, b: nc.gpsimd.tensor_tensor(o, a, b, OP.add)

                # cofactor/Cramer terms. M3/M5/det run on GpSimd (idle
                # capacity); the rest on DVE.
                sq12 = tmp("sq12")
                nc.scalar.square(sq12[:], s12[:])
                def tmp1(tag):
                    return sv.tile([TILE_H, CW], F32, tag=tag, name=tag, bufs=1)

                ta = tmp1("ta")
                tb = tmp1("tb")
                ga = tmp1("ga")
                gb = tmp1("gb")
                M1 = tmp("M1")
                mul(ta[:], s11[:], s22[:])
                sub(M1[:], ta[:], sq12[:])
                M2 = tmp("M2")
                mul(ta[:], b1[:], s22[:])
                mul(tb[:], s12[:], b2[:])
                sub(M2[:], ta[:], tb[:])
                M3 = tmp("M3")
                gmul(ga[:], b1[:], s12[:])
                gmul(gb[:], s11[:], b2[:])
                gsub(M3[:], ga[:], gb[:])
                M4 = tmp("M4")
                mul(ta[:], s01[:], s22[:])
                mul(tb[:], s02[:], s12[:])
                sub(M4[:], ta[:], tb[:])
                M5 = tmp("M5")
                gmul(ga[:], s01[:], b2[:])
                gmul(gb[:], s02[:], b1[:])
                gsub(M5[:], ga[:], gb[:])
                M6 = tmp("M6")
                mul(ta[:], s01[:], s12[:])
                mul(tb[:], s02[:], s11[:])
                sub(M6[:], ta[:], tb[:])

                # det = s00*M1 - s01*M4 + s02*M6 (GpSimd; only feeds `good`)
                det = tmp("det")
                gmul(ga[:], s00[:], M1[:])
                gmul(gb[:], s01[:], M4[:])
                gsub(det[:], ga[:], gb[:])
                gmul(ga[:], s02[:], M6[:])
                gadd(det[:], det[:], ga[:])

                # Cramer numerators (DVE)
                d0 = tmp("d0")
                mul(ta[:], b0[:], M1[:])
                mul(tb[:], s01[:], M2[:])
                sub(d0[:], ta[:], tb[:])
                mul(ta[:], s02[:], M3[:])
                add(d0[:], d0[:], ta[:])
                d1 = tmp("d1")
                mul(ta[:], s00[:], M2[:])
                mul(tb[:], b0[:], M4[:])
                sub(d1[:], ta[:], tb[:])
                mul(ta[:], s02[:], M5[:])
                add(d1[:], d1[:], ta[:])
                d2 = tmp("d2")
                mul(ta[:], b0[:], M6[:])
                mul(tb[:], s00[:], M3[:])
                sub(d2[:], ta[:], tb[:])
                mul(ta[:], s01[:], M5[:])
                sub(d2[:], d2[:], ta[:])

                # good = det >= DET_EPS (catches NaN too: NaN>=eps is false).
                # Good pixels: n ∝ d_i (1/det > 0 scale dropped — normalize
                # makes it irrelevant up to the +eps in the denominator, which
                # is far below this problem's f32 conditioning noise).
                # Bad pixels: n ∝ Atb (AtA replaced by I).
                good = sv.tile([TILE_H, CW], mybir.dt.uint8, tag="good", name="good")
                nc.vector.tensor_scalar(good[:], det[:], DET_EPS, None, op0=OP.is_ge)
                m = []
                qs = []
                for mi, bi, di in ((0, b0, d0), (1, b1, d1), (2, b2, d2)):
                    m_i = tmp(f"m{mi}")
                    nc.scalar.copy(m_i[:], bi[:])
                    nc.vector.copy_predicated(m_i[:], good[:], di[:])
                    m.append(m_i)
                    q_i = tmp(f"q{mi}")
                    nc.scalar.square(q_i[:], m_i[:])
                    qs.append(q_i)

                # normalize: out_i = m_i / (||m|| + NORM_EPS)
                q = tmp("q")
                add(q[:], qs[0][:], qs[1][:])
                add(q[:], q[:], qs[2][:])
                sn = tmp("sn")
                nc.scalar.sqrt(sn[:], q[:])
                nc.vector.tensor_scalar_add(sn[:], sn[:], NORM_EPS)
                rn = tmp("rn")
                nc.vector.reciprocal_approx_fast(rn[:], sn[:])
                for i in range(3):
                    out_i = tmp(f"out{i}")
                    mul(out_i[:], m[i][:], rn[:])
                    nc.sync.dma_start(
                        d_norm[i, t * TILE_H : (t + 1) * TILE_H, j0 : j0 + CW],
                        out_i[:],
                    )


def _host_constants(Kinv):
    """Per-core constant tensors derived from one [3,3] inverse intrinsic."""
    Ki = Kinv.astype(np.float32)
    band = np.zeros((128, TILE_H), np.float32)
    for k in range(128):
        lo, hi = max(0, k - 2 * HALO), min(TILE_H - 1, k)
        if lo <= hi:
            band[k, lo : hi + 1] = 1.0
    xg = np.tile((np.arange(PW, dtype=np.float32) - HALO)[None, :], (128, 1))
    ysc = np.zeros((128, 3 * N_TILES), np.float32)
    p = np.arange(128, dtype=np.float32)
    for t in range(N_TILES):
        hrow = t * TILE_H - HALO + p
        for i in range(3):
            ysc[:, 3 * t + i] = Ki[i, 1] * hrow + Ki[i, 2]
    kx = np.tile(Ki[:, 0][None, :], (128, 1)).astype(np.float32)
    return {"consts": np.concatenate([band, ysc, kx, xg], axis=1)}


_PROGRAM = None


def _program():
    global _PROGRAM
    if _PROGRAM is None:
        _PROGRAM = _build_program()
    return _PROGRAM


def _run(depth, intrinsic_inv, trace=False, **kw):
    nc = _program()
    in_maps = []
    for b in range(B):
        m = {"depth": np.ascontiguousarray(depth[b], dtype=np.float32)}
        m.update(_host_constants(np.asarray(intrinsic_inv[b])))
        in_maps.append(m)
    return run_bass_kernel_spmd(nc, in_maps, core_ids=list(range(B)), trace=trace, **kw)


def kernel(depth, intrinsic_inv):
    res = _run(np.asarray(depth), np.asarray(intrinsic_inv), trace=False)
    normals = np.stack([res.results[b]["normals"] for b in range(B)])
    points = np.stack([res.results[b]["points"] for b in range(B)])
    return normals.astype(np.float32), points.astype(np.float32)


# revision 20
# speedup vs baseline: 1.5914x; 1.4801x over previous
"""Depth2Normal Trainium2 kernel.

Computes, per batch image: camera-space points from depth + intrinsics, and
per-pixel surface normals via a 9x9-window least-squares plane fit
(AtA n = Atb solved with Cramer's rule; singular windows fall back to Atb),
matching reference.py semantics.

Sharding: pure data parallel, one batch element per NeuronCore (B=8 = 8 cores).

Per-core layout: partitions = H rows (4 tiles of 120 output rows + 4 halo on
each side = 128), free dim = W (640 + 4 zero-pad each side = 648).

Box sums: both window directions run on the TensorEngine. For each output
W-chunk, 9 accumulating matmuls (one per W-shift s in 0..8) with a banded
[128,120] ones matrix as the stationary operand compute the full 2D 9x9 sum
into PSUM. ScalarE evacuates PSUM->SBUF; the per-pixel 3x3 solve runs on the
VectorEngine.
"""

import sys

sys.path.insert(0, "/opt/trn_rl_repo")

import numpy as np

from concourse import bacc, bass, mybir, tile
from concourse.bass_utils import run_bass_kernel_spmd

F32 = mybir.dt.float32
AF = mybir.ActivationFunctionType
OP = mybir.AluOpType

B, H, W = 8, 480, 640
K_SIZE = 9
HALO = K_SIZE // 2  # 4
TILE_H = 120  # output rows per H-tile
N_TILES = H // TILE_H  # 4
PW = W + 2 * HALO  # padded width 648
CW = 320  # W-chunk width (PSUM bank holds 512 f32; 320*4B=1280B)
N_CHUNKS = W // CW  # 2
DEPTH_MAX = 10.0
DET_EPS = 1e-5
NORM_EPS = 1e-5


def _build_program(n_cores=8):
    nc = bacc.Bacc(
        "TRN2",
        target_bir_lowering=False,
        debug=False,
        enable_asserts=False,
        num_devices=n_cores,
    )
    d_depth = nc.dram_tensor("depth", [H, W], F32, kind="ExternalInput").ap()
    # consts layout: [0:120]=band, [120:132]=ysc, [132:135]=kx, [135:135+PW]=xgrid
    d_const = nc.dram_tensor(
        "consts", [128, TILE_H + 3 * N_TILES + 3 + PW], F32, kind="ExternalInput"
    ).ap()
    d_norm = nc.dram_tensor("normals", [3, H, W], F32, kind="ExternalOutput").ap()
    d_pts = nc.dram_tensor("points", [3, H, W], F32, kind="ExternalOutput").ap()

    with tile.TileContext(nc) as tc:
        _body(tc, d_depth, d_const, d_norm, d_pts)
    nc.compile()
    return nc


def _chap(base, slot, ch0, chstep, nch, cw, col0=0):
    """Multi-channel strided view into a [P, n*slot] tile: channels
    ch0, ch0+chstep, ... each [cw] wide starting at col0."""
    v = base[:, ch0 * slot + col0 : ch0 * slot + col0 + cw].copy()
    v.ap = mybir.VecI64Pair([list(v.ap[0]), [chstep * slot, nch], [1, cw]])
    return v


def _body(tc, d_depth, d_const, d_norm, d_pts):
    nc = tc.nc
    from contextlib import ExitStack

    TT = nc.vector.tensor_tensor
    PWS = PW  # 648 pointwise slot width
    NW = CW + K_SIZE - 1  # 328 box-window input cols per chunk
    SLOT = NW  # solve-tile slot width

    with ExitStack() as ctx:
        const = ctx.enter_context(tc.tile_pool(name="const", bufs=1))
        io = ctx.enter_context(tc.tile_pool(name="io", bufs=1))
        chp = ctx.enter_context(tc.tile_pool(name="ch", bufs=2))
        ps = ctx.enter_context(tc.tile_pool(name="ps", bufs=8, space="PSUM"))
        sv = ctx.enter_context(tc.tile_pool(name="sv", bufs=1))
        sv2 = ctx.enter_context(tc.tile_pool(name="sv2", bufs=2))

        # --- constants (one DMA; slice views) ---
        NCOLS = TILE_H + 3 * N_TILES + 3 + PW
        ct = const.tile([128, NCOLS], F32)
        nc.sync.dma_start(ct[:], d_const[:])
        band = ct[:, 0:TILE_H]
        ysc = ct[:, TILE_H : TILE_H + 3 * N_TILES]
        kx = ct[:, TILE_H + 3 * N_TILES : TILE_H + 3 * N_TILES + 3]
        xgb = ct[:, TILE_H + 3 * N_TILES + 3 : NCOLS]

        # persistent shifted-window operands for the box scan (cols 0..8 stay 0)
        zt = []
        for ci in range(9):
            z = const.tile([TILE_H, NW], F32, tag=f"z{ci}", name=f"z{ci}")
            nc.gpsimd.memset(z[:, 0:K_SIZE], 0.0)
            zt.append(z)

        for t in range(N_TILES):
            # --- DD: slot0 = depth (zero-padded), slot1 = dmask ---
            dd = io.tile([128, 2 * PWS], F32, tag="dd", name="dd")
            nc.gpsimd.memset(dd[:, 0:PWS], 0.0)
            rows0 = t * TILE_H - HALO
            r_lo = max(rows0, 0)
            r_hi = min(rows0 + 128, H)
            p_lo = r_lo - rows0
            nc.sync.dma_start(
                dd[p_lo : p_lo + (r_hi - r_lo), HALO : HALO + W], d_depth[r_lo:r_hi, :]
            )
            dep = dd[:, 0:PWS]
            # dmask = depth * (depth<10) * (depth>0), two fused stt ops
            td = io.tile([128, PWS], F32, tag="td", name="td")
            nc.vector.scalar_tensor_tensor(
                td[:], dep, DEPTH_MAX, dep, op0=OP.is_lt, op1=OP.mult
            )
            nc.vector.scalar_tensor_tensor(
                dd[:, PWS : 2 * PWS], dep, 0.0, td[:], op0=OP.is_gt, op1=OP.mult
            )

            # --- P_i then (mp_i, cam_i) dual products ---
            # CH slots: 0-5 = outer products s00..s22, 6-8 = mp, 9-11 = cam
            p3 = io.tile([128, 3 * PWS], F32, tag="p3", name="p3")
            ch = chp.tile([128, 12 * PWS], F32, tag="ch", name="ch")
            for i in range(3):
                nc.scalar.activation(
                    p3[:, i * PWS : (i + 1) * PWS],
                    xgb,
                    AF.Identity,
                    bias=ysc[:, 3 * t + i : 3 * t + i + 1],
                    scale=kx[:, i : i + 1],
                )
                # out (mp_i@6+i, cam_i@9+i), in1 = (dmask@1, depth@0)
                TT(
                    _chap(ch, PWS, 6 + i, 3, 2, PWS),
                    _chap(p3, PWS, i, 0, 2, PWS),
                    _chap(dd, PWS, 1, -1, 2, PWS),
                    OP.mult,
                )
                nc.sync.dma_start(
                    d_pts[i, t * TILE_H : (t + 1) * TILE_H, :],
                    ch[HALO : HALO + TILE_H, (9 + i) * PWS + HALO : (9 + i) * PWS + HALO + W],
                )
            # outer products: diag via ACT Square, off-diag via dual + single
            for i, s in ((0, 0), (1, 3), (2, 5)):
                nc.scalar.square(
                    ch[:, s * PWS : s * PWS + PWS], ch[:, (6 + i) * PWS : (7 + i) * PWS]
                )
            TT(
                _chap(ch, PWS, 1, 1, 2, PWS),
                _chap(ch, PWS, 6, 0, 2, PWS),
                _chap(ch, PWS, 7, 1, 2, PWS),
                OP.mult,
            )
            TT(
                ch[:, 4 * PWS : 5 * PWS],
                ch[:, 7 * PWS : 8 * PWS],
                ch[:, 8 * PWS : 9 * PWS],
                OP.mult,
            )

            for c in range(N_CHUNKS):
                j0 = c * CW
                # --- box sums: H-band matmul -> PSUM; fused running-difference
                # scan (cumsum minus 9-delayed cumsum) -> BX channel ---
                bx = sv2.tile([TILE_H, 9 * SLOT], F32, tag="bx", name="bx")
                for ci in range(9):
                    pt = ps.tile([TILE_H, NW], F32, tag="ps", name="ps")
                    nc.tensor.matmul(
                        pt[:], band, ch[:, ci * PWS + j0 : ci * PWS + j0 + NW],
                        start=True, stop=True,
                    )
                    nc.scalar.copy(zt[ci][:, K_SIZE:NW], pt[:, 0 : NW - K_SIZE])
                    nc.vector.tensor_tensor_scan(
                        bx[:, ci * SLOT : ci * SLOT + NW],
                        pt[:],
                        zt[ci][:],
                        0.0,
                        OP.add,
                        OP.subtract,
                    )
                # box value for output col j lives at BX col j+8
                BOX0 = K_SIZE - 1  # box[j] lives at bx col j+8
                B = lambda ch0, chstep, nch: _chap(bx, SLOT, ch0, chstep, nch, CW, col0=BOX0)
                # BX channels: s00=0 s01=1 s02=2 s11=3 s12=4 s22=5 b0=6 b1=7 b2=8

                # --- cofactor products PR: PA=0-5, PB=6-11; M = PA - PB ---
                pr = sv.tile([TILE_H, 12 * SLOT], F32, tag="pr", name="pr")
                P = lambda ch0, chstep, nch: _chap(pr, SLOT, ch0, chstep, nch, CW)
                TT(P(3, 6, 2), B(1, 1, 2), B(5, -1, 2), OP.mult)   # s01*s22, s02*s12
                TT(P(5, 6, 2), B(1, 1, 2), B(4, -1, 2), OP.mult)   # s01*s12, s02*s11
                TT(P(4, 6, 2), B(1, 1, 2), B(8, -1, 2), OP.mult)   # s01*b2,  s02*b1
                TT(P(1, 6, 2), B(7, -3, 2), B(5, 3, 2), OP.mult)   # b1*s22,  s12*b2
                TT(P(0, 8, 2), B(3, 0, 2), B(5, 3, 2), OP.mult)    # s11*s22, s11*b2
                TT(P(2, 1, 1), B(7, 1, 1), B(4, 1, 1), OP.mult)    # b1*s12
                nc.scalar.activation(
                    pr[:, 6 * SLOT : 6 * SLOT + CW],
                    bx[:, 4 * SLOT + BOX0 : 4 * SLOT + BOX0 + CW],
                    AF.Square,
                )  # s12^2
                m6 = sv.tile([TILE_H, 6 * SLOT], F32, tag="m6", name="m6")
                M = lambda ch0, chstep, nch: _chap(m6, SLOT, ch0, chstep, nch, CW)
                TT(M(0, 1, 6), P(0, 1, 6), P(6, 1, 6), OP.subtract)
                # M slots: M1=0 M2=1 M3=2 M4=3 M5=4 M6=5

                # --- det/d products DP: A=[s00M1,b0M1,s00M2,b0M6] 0-3,
                # B=[s01M4,s01M2,b0M4,s00M3] 4-7, C=[s02M6,s02M3,s02M5,s01M5] 8-11
                dp = sv.tile([TILE_H, 12 * SLOT], F32, tag="dp", name="dp")
                D = lambda ch0, chstep, nch: _chap(dp, SLOT, ch0, chstep, nch, CW)
                TT(D(0, 2, 2), B(0, 0, 2), M(0, 1, 2), OP.mult)    # s00M1, s00M2
                TT(D(7, 1, 1), B(0, 1, 1), M(2, 1, 1), OP.mult)    # s00M3
                TT(D(4, 1, 2), B(1, 0, 2), M(3, -2, 2), OP.mult)   # s01M4, s01M2
                TT(D(11, 1, 1), B(1, 1, 1), M(4, 1, 1), OP.mult)   # s01M5
                TT(D(8, 1, 2), B(2, 0, 2), M(5, -3, 2), OP.mult)   # s02M6, s02M3
                TT(D(10, 1, 1), B(2, 1, 1), M(4, 1, 1), OP.mult)   # s02M5
                TT(D(1, 5, 2), B(6, 0, 2), M(0, 3, 2), OP.mult)    # b0M1, b0M4
                TT(D(3, 1, 1), B(6, 1, 1), M(5, 1, 1), OP.mult)    # b0M6
                # S = A - B (+C / -C): channels (det, d0, d1, d2)
                st = sv.tile([TILE_H, 4 * SLOT], F32, tag="st", name="st")
                S = lambda ch0, chstep, nch: _chap(st, SLOT, ch0, chstep, nch, CW)
                TT(S(0, 1, 4), D(0, 1, 4), D(4, 1, 4), OP.subtract)
                s2 = sv2.tile([TILE_H, 4 * SLOT], F32, tag="s2", name="s2")
                S2 = lambda ch0, chstep, nch: _chap(s2, SLOT, ch0, chstep, nch, CW)
                TT(S2(0, 1, 3), S(0, 1, 3), D(8, 1, 3), OP.add)
                TT(S2(3, 1, 1), S(3, 1, 1), D(11, 1, 1), OP.subtract)

                # --- good = det >= eps; blend m = good ? d : Atb; normalize ---
                good = sv2.tile(
                    [TILE_H, CW], mybir.dt.uint8, tag="good", name="good"
                )
                nc.vector.tensor_scalar(
                    good[:], s2[:, 0:CW], DET_EPS, None, op0=OP.is_ge
                )
                m3 = sv.tile([TILE_H, 3 * SLOT], F32, tag="m3", name="m3")
                for i in range(3):
                    nc.scalar.copy(
                        m3[:, i * SLOT : i * SLOT + CW],
                        bx[:, (6 + i) * SLOT + BOX0 : (6 + i) * SLOT + BOX0 + CW],
                    )
                gv = good[:, 0:CW].copy()
                gv.ap = mybir.VecI64Pair([list(gv.ap[0]), [0, 3], [1, CW]])
                nc.vector.copy_predicated(
                    _chap(m3, SLOT, 0, 1, 3, CW), gv, S2(1, 1, 3)
                )
                qt = sv.tile([TILE_H, 3 * SLOT], F32, tag="qt", name="qt")
                for i in range(3):
                    nc.scalar.square(
                        qt[:, i * SLOT : i * SLOT + CW], m3[:, i * SLOT : i * SLOT + CW]
                    )
                q = sv.tile([TILE_H, CW], F32, tag="q", name="q")
                TT(q[:], qt[:, 0:CW], qt[:, SLOT : SLOT + CW], OP.add)
                q2 = sv.tile([TILE_H, CW], F32, tag="q2", name="q2")
                TT(q2[:], q[:], qt[:, 2 * SLOT : 2 * SLOT + CW], OP.add)
                sn = sv.tile([TILE_H, CW], F32, tag="sn", name="sn")
                nc.scalar.sqrt(sn[:], q2[:])
                sn2 = sv.tile([TILE_H, CW], F32, tag="sn2", name="sn2")
                nc.vector.tensor_scalar_add(sn2[:], sn[:], NORM_EPS)
                rn = sv.tile([TILE_H, CW], F32, tag="rn", name="rn")
                nc.vector.reciprocal_approx_fast(rn[:], sn2[:])
                o3 = sv2.tile([TILE_H, 3 * SLOT], F32, tag="o3", name="o3")
                rv = rn[:, 0:CW].copy()
                rv.ap = mybir.VecI64Pair([list(rv.ap[0]), [0, 3], [1, CW]])
                TT(_chap(o3, SLOT, 0, 1, 3, CW), _chap(m3, SLOT, 0, 1, 3, CW), rv, OP.mult)
                for i in range(3):
                    nc.sync.dma_start(
                        d_norm[i, t * TILE_H : (t + 1) * TILE_H, j0 : j0 + CW],
                        o3[:, i * SLOT : i * SLOT + CW],
                    )


def _host_constants(Kinv):
    """Per-core constant tensors derived from one [3,3] inverse intrinsic."""
    Ki = Kinv.astype(np.float32)
    band = np.zeros((128, TILE_H), np.float32)
    for k in range(128):
        lo, hi = max(0, k - 2 * HALO), min(TILE_H - 1, k)
        if lo <= hi:
            band[k, lo : hi + 1] = 1.0
    xg = np.tile((np.arange(PW, dtype=np.float32) - HALO)[None, :], (128, 1))
    ysc = np.zeros((128, 3 * N_TILES), np.float32)
    p = np.arange(128, dtype=np.float32)
    for t in range(N_TILES):
        hrow = t * TILE_H - HALO + p
        for i in range(3):
            ysc[:, 3 * t + i] = Ki[i, 1] * hrow + Ki[i, 2]
    kx = np.tile(Ki[:, 0][None, :], (128, 1)).astype(np.float32)
    return {"consts": np.concatenate([band, ysc, kx, xg], axis=1)}


_PROGRAM = None


def _program():
    global _PROGRAM
    if _PROGRAM is None:
        _PROGRAM = _build_program()
    return _PROGRAM


def _run(depth, intrinsic_inv, trace=False, **kw):
    nc = _program()
    in_maps = []
    for b in range(B):
        m = {"depth": np.ascontiguousarray(depth[b], dtype=np.float32)}
        m.update(_host_constants(np.asarray(intrinsic_inv[b])))
        in_maps.append(m)
    return run_bass_kernel_spmd(nc, in_maps, core_ids=list(range(B)), trace=trace, **kw)


def kernel(depth, intrinsic_inv):
    res = _run(np.asarray(depth), np.asarray(intrinsic_inv), trace=False)
    normals = np.stack([res.results[b]["normals"] for b in range(B)])
    points = np.stack([res.results[b]["points"] for b in range(B)])
    return normals.astype(np.float32), points.astype(np.float32)


# revision 24
# speedup vs baseline: 1.6556x; 1.0403x over previous
"""Depth2Normal Trainium2 kernel.

Computes, per batch image: camera-space points from depth + intrinsics, and
per-pixel surface normals via a 9x9-window least-squares plane fit
(AtA n = Atb solved with Cramer's rule; singular windows fall back to Atb),
matching reference.py semantics.

Sharding: pure data parallel, one batch element per NeuronCore (B=8 = 8 cores).

Per-core layout: partitions = H rows (4 tiles of 120 output rows + 4 halo on
each side = 128), free dim = W (640 + 4 zero-pad each side = 648).

Box sums: both window directions run on the TensorEngine. For each output
W-chunk, 9 accumulating matmuls (one per W-shift s in 0..8) with a banded
[128,120] ones matrix as the stationary operand compute the full 2D 9x9 sum
into PSUM. ScalarE evacuates PSUM->SBUF; the per-pixel 3x3 solve runs on the
VectorEngine.
"""

import sys

sys.path.insert(0, "/opt/trn_rl_repo")

import numpy as np

from concourse import bacc, bass, mybir, tile
from concourse.bass_utils import run_bass_kernel_spmd

F32 = mybir.dt.float32
AF = mybir.ActivationFunctionType
OP = mybir.AluOpType

B, H, W = 8, 480, 640
K_SIZE = 9
HALO = K_SIZE // 2  # 4
TILE_H = 120  # output rows per H-tile
N_TILES = H // TILE_H  # 4
PW = W + 2 * HALO  # padded width 648
CW = 320  # W-chunk width (PSUM bank holds 512 f32; 320*4B=1280B)
N_CHUNKS = W // CW  # 2
DEPTH_MAX = 10.0
DET_EPS = 1e-5
NORM_EPS = 1e-5


def _build_program(n_cores=8):
    nc = bacc.Bacc(
        "TRN2",
        target_bir_lowering=False,
        debug=False,
        enable_asserts=False,
        num_devices=n_cores,
    )
    d_depth = nc.dram_tensor("depth", [H, W], F32, kind="ExternalInput").ap()
    # consts layout: [0:120]=band, [120:132]=ysc, [132:135]=kx, [135:135+PW]=xgrid
    d_const = nc.dram_tensor(
        "consts", [128, TILE_H + 3 * N_TILES + 3 + PW], F32, kind="ExternalInput"
    ).ap()
    d_norm = nc.dram_tensor("normals", [3, H, W], F32, kind="ExternalOutput").ap()
    d_pts = nc.dram_tensor("points", [3, H, W], F32, kind="ExternalOutput").ap()

    with tile.TileContext(nc) as tc:
        _body(tc, d_depth, d_const, d_norm, d_pts)
    nc.compile()
    return nc


def _chap(base, slot, ch0, chstep, nch, cw, col0=0):
    """Multi-channel strided view into a [P, n*slot] tile: channels
    ch0, ch0+chstep, ... each [cw] wide starting at col0."""
    v = base[:, ch0 * slot + col0 : ch0 * slot + col0 + cw].copy()
    v.ap = mybir.VecI64Pair([list(v.ap[0]), [chstep * slot, nch], [1, cw]])
    return v


def _body(tc, d_depth, d_const, d_norm, d_pts):
    nc = tc.nc
    from contextlib import ExitStack

    TT = nc.vector.tensor_tensor
    pt_out = lambda p: p[:]
    PWS = PW  # 648 pointwise slot width
    NW = CW + K_SIZE - 1  # 328 box-window input cols per chunk
    SLOT = NW  # solve-tile slot width

    with ExitStack() as ctx:
        const = ctx.enter_context(tc.tile_pool(name="const", bufs=1))
        io = ctx.enter_context(tc.tile_pool(name="io", bufs=1))
        chp = ctx.enter_context(tc.tile_pool(name="ch", bufs=2))
        ps = ctx.enter_context(tc.tile_pool(name="ps", bufs=8, space="PSUM"))
        sv = ctx.enter_context(tc.tile_pool(name="sv", bufs=1))
        sv2 = ctx.enter_context(tc.tile_pool(name="sv2", bufs=2))

        # --- constants (one DMA; slice views) ---
        NCOLS = TILE_H + 3 * N_TILES + 3 + PW
        ct = const.tile([128, NCOLS], F32)
        nc.sync.dma_start(ct[:], d_const[:])
        band = ct[:, 0:TILE_H]
        ysc = ct[:, TILE_H : TILE_H + 3 * N_TILES]
        kx = ct[:, TILE_H + 3 * N_TILES : TILE_H + 3 * N_TILES + 3]
        xgb = ct[:, TILE_H + 3 * N_TILES + 3 : NCOLS]

        # persistent shifted-window operands for the box scan (cols 0..8 stay 0)
        zt = []
        for ci in range(9):
            z = const.tile([TILE_H, K_SIZE + PW, ], F32, tag=f"z{ci}", name=f"z{ci}")
            nc.gpsimd.memset(z[:, 0:K_SIZE], 0.0)
            zt.append(z)

        for t in range(N_TILES):
            # --- DD: slot0 = depth (zero-padded), slot1 = dmask ---
            dd = io.tile([128, 2 * PWS], F32, tag="dd", name="dd")
            nc.gpsimd.memset(dd[:, 0:PWS], 0.0)
            rows0 = t * TILE_H - HALO
            r_lo = max(rows0, 0)
            r_hi = min(rows0 + 128, H)
            p_lo = r_lo - rows0
            nc.sync.dma_start(
                dd[p_lo : p_lo + (r_hi - r_lo), HALO : HALO + W], d_depth[r_lo:r_hi, :]
            )
            dep = dd[:, 0:PWS]
            # dmask = depth * (depth<10) * (depth>0), two fused stt ops
            td = io.tile([128, PWS], F32, tag="td", name="td")
            nc.vector.scalar_tensor_tensor(
                td[:], dep, DEPTH_MAX, dep, op0=OP.is_lt, op1=OP.mult
            )
            nc.vector.scalar_tensor_tensor(
                dd[:, PWS : 2 * PWS], dep, 0.0, td[:], op0=OP.is_gt, op1=OP.mult
            )

            # --- P_i then (mp_i, cam_i) dual products ---
            # CH slots: 0-5 = outer products s00..s22, 6-8 = mp, 9-11 = cam
            p3 = io.tile([128, 3 * PWS], F32, tag="p3", name="p3")
            ch = chp.tile([128, 12 * PWS], F32, tag="ch", name="ch", bufs=1)
            for i in range(3):
                nc.scalar.activation(
                    p3[:, i * PWS : (i + 1) * PWS],
                    xgb,
                    AF.Identity,
                    bias=ysc[:, 3 * t + i : 3 * t + i + 1],
                    scale=kx[:, i : i + 1],
                )
                # out (mp_i@6+i, cam_i@9+i), in1 = (dmask@1, depth@0)
                TT(
                    _chap(ch, PWS, 6 + i, 3, 2, PWS),
                    _chap(p3, PWS, i, 0, 2, PWS),
                    _chap(dd, PWS, 1, -1, 2, PWS),
                    OP.mult,
                )
                nc.sync.dma_start(
                    d_pts[i, t * TILE_H : (t + 1) * TILE_H, :],
                    ch[HALO : HALO + TILE_H, (9 + i) * PWS + HALO : (9 + i) * PWS + HALO + W],
                )
            # outer products: diag via ACT Square, off-diag via dual + single
            for i, s in ((0, 0), (1, 3), (2, 5)):
                nc.scalar.square(
                    ch[:, s * PWS : s * PWS + PWS], ch[:, (6 + i) * PWS : (7 + i) * PWS]
                )
            TT(
                _chap(ch, PWS, 1, 1, 2, PWS),
                _chap(ch, PWS, 6, 0, 2, PWS),
                _chap(ch, PWS, 7, 1, 2, PWS),
                OP.mult,
            )
            TT(
                ch[:, 4 * PWS : 5 * PWS],
                ch[:, 7 * PWS : 8 * PWS],
                ch[:, 8 * PWS : 9 * PWS],
                OP.mult,
            )

            # --- full-width box sums + solve (one pass, cw=640) ---
            # H-band matmuls (N<=512 forces a 512+136 split) -> PSUM; ScalarE
            # assembles PZ = [9 zeros | P(648)] in SBUF; one DVE scan per
            # channel computes the 9-wide running difference (2D box sum).
            WIN = W + K_SIZE - 1  # 648 window input cols
            SB = WIN  # BX slot
            S2W = W  # solve slot
            bx = sv2.tile([TILE_H, 9 * SB], F32, tag="bx", name="bx", bufs=1)
            for ci in range(9):
                pa = ps.tile([TILE_H, 512], F32, tag="pa", name="pa", bufs=4)
                nc.tensor.matmul(
                    pt_out(pa), band, ch[:, ci * PWS : ci * PWS + 512],
                    start=True, stop=True,
                )
                pb = ps.tile([TILE_H, WIN - 512], F32, tag="pb", name="pb", bufs=4)
                nc.tensor.matmul(
                    pt_out(pb), band, ch[:, ci * PWS + 512 : ci * PWS + WIN],
                    start=True, stop=True,
                )
                nc.scalar.copy(zt[ci][:, K_SIZE : K_SIZE + 512], pa[:])
                nc.scalar.copy(zt[ci][:, K_SIZE + 512 : K_SIZE + WIN], pb[:])
                nc.vector.tensor_tensor_scan(
                    bx[:, ci * SB : ci * SB + WIN],
                    zt[ci][:, K_SIZE : K_SIZE + WIN],
                    zt[ci][:, 0:WIN],
                    0.0,
                    OP.add,
                    OP.subtract,
                )
            BOX0 = K_SIZE - 1  # box[j] lives at bx col j+8
            B = lambda ch0, chstep, nch: _chap(bx, SB, ch0, chstep, nch, S2W, col0=BOX0)
            # BX channels: s00=0 s01=1 s02=2 s11=3 s12=4 s22=5 b0=6 b1=7 b2=8

            # --- cofactor products PR: PA=0-5, PB=6-11; M = PA - PB ---
            pr = sv.tile([TILE_H, 12 * S2W], F32, tag="work12", name="pr")
            P = lambda ch0, chstep, nch: _chap(pr, S2W, ch0, chstep, nch, S2W)
            TT(P(3, 6, 2), B(1, 1, 2), B(5, -1, 2), OP.mult)   # s01*s22, s02*s12
            TT(P(5, 6, 2), B(1, 1, 2), B(4, -1, 2), OP.mult)   # s01*s12, s02*s11
            TT(P(4, 6, 2), B(1, 1, 2), B(8, -1, 2), OP.mult)   # s01*b2,  s02*b1
            TT(P(1, 6, 2), B(7, -3, 2), B(5, 3, 2), OP.mult)   # b1*s22,  s12*b2
            TT(P(0, 8, 2), B(3, 0, 2), B(5, 3, 2), OP.mult)    # s11*s22, s11*b2
            TT(P(2, 1, 1), B(7, 1, 1), B(4, 1, 1), OP.mult)    # b1*s12
            nc.scalar.activation(
                pr[:, 6 * S2W : 7 * S2W],
                bx[:, 4 * SB + BOX0 : 4 * SB + BOX0 + S2W],
                AF.Square,
            )  # s12^2
            m6 = sv.tile([TILE_H, 6 * S2W], F32, tag="m6", name="m6")
            M = lambda ch0, chstep, nch: _chap(m6, S2W, ch0, chstep, nch, S2W)
            TT(M(0, 1, 6), P(0, 1, 6), P(6, 1, 6), OP.subtract)
            # M slots: M1=0 M2=1 M3=2 M4=3 M5=4 M6=5

            # --- det/d products DP (same slot as PR, freed by the M-sub):
            # A=[s00M1,b0M1,s00M2,b0M6] 0-3, B=[s01M4,s01M2,b0M4,s00M3] 4-7,
            # C=[s02M6,s02M3,s02M5,s01M5] 8-11
            dp = sv.tile([TILE_H, 12 * S2W], F32, tag="work12", name="dp")
            D = lambda ch0, chstep, nch: _chap(dp, S2W, ch0, chstep, nch, S2W)
            TT(D(0, 2, 2), B(0, 0, 2), M(0, 1, 2), OP.mult)    # s00M1, s00M2
            TT(D(7, 1, 1), B(0, 1, 1), M(2, 1, 1), OP.mult)    # s00M3
            TT(D(4, 1, 2), B(1, 0, 2), M(3, -2, 2), OP.mult)   # s01M4, s01M2
            TT(D(11, 1, 1), B(1, 1, 1), M(4, 1, 1), OP.mult)   # s01M5
            TT(D(8, 1, 2), B(2, 0, 2), M(5, -3, 2), OP.mult)   # s02M6, s02M3
            TT(D(10, 1, 1), B(2, 1, 1), M(4, 1, 1), OP.mult)   # s02M5
            TT(D(1, 5, 2), B(6, 0, 2), M(0, 3, 2), OP.mult)    # b0M1, b0M4
            TT(D(3, 1, 1), B(6, 1, 1), M(5, 1, 1), OP.mult)    # b0M6
            st = sv.tile([TILE_H, 4 * S2W], F32, tag="st", name="st")
            S = lambda ch0, chstep, nch: _chap(st, S2W, ch0, chstep, nch, S2W)
            TT(S(0, 1, 4), D(0, 1, 4), D(4, 1, 4), OP.subtract)
            s2 = sv2.tile([TILE_H, 4 * S2W], F32, tag="s2", name="s2", bufs=1)
            S2 = lambda ch0, chstep, nch: _chap(s2, S2W, ch0, chstep, nch, S2W)
            TT(S2(0, 1, 3), S(0, 1, 3), D(8, 1, 3), OP.add)
            TT(S2(3, 1, 1), S(3, 1, 1), D(11, 1, 1), OP.subtract)

            # --- good = det >= eps; blend m = good ? d : Atb; normalize ---
            good = sv2.tile([TILE_H, S2W], mybir.dt.uint8, tag="good", name="good")
            nc.vector.tensor_scalar(good[:], s2[:, 0:S2W], DET_EPS, None, op0=OP.is_ge)
            m3 = sv.tile([TILE_H, 3 * S2W], F32, tag="st", name="m3")
            for i in range(3):
                nc.scalar.copy(
                    m3[:, i * S2W : (i + 1) * S2W],
                    bx[:, (6 + i) * SB + BOX0 : (6 + i) * SB + BOX0 + S2W],
                )
            for i in range(3):
                nc.vector.copy_predicated(
                    m3[:, i * S2W : (i + 1) * S2W], good[:], s2[:, (1 + i) * S2W : (2 + i) * S2W]
                )
            qt = sv.tile([TILE_H, 3 * S2W], F32, tag="m6", name="qt")
            for i in range(3):
                nc.scalar.square(
                    qt[:, i * S2W : (i + 1) * S2W], m3[:, i * S2W : (i + 1) * S2W]
                )
            q = sv.tile([TILE_H, S2W], F32, tag="q", name="q")
            TT(q[:], qt[:, 0:S2W], qt[:, S2W : 2 * S2W], OP.add)
            q2 = sv.tile([TILE_H, S2W], F32, tag="q2", name="q2")
            TT(q2[:], q[:], qt[:, 2 * S2W : 3 * S2W], OP.add)
            sn = sv.tile([TILE_H, S2W], F32, tag="sn", name="sn")
            nc.scalar.sqrt(sn[:], q2[:])
            sn2 = sv.tile([TILE_H, S2W], F32, tag="sn2", name="sn2")
            nc.vector.tensor_scalar_add(sn2[:], sn[:], NORM_EPS)
            rn = sv.tile([TILE_H, S2W], F32, tag="rn", name="rn")
            nc.vector.reciprocal_approx_fast(rn[:], sn2[:])
            o3 = sv2.tile([TILE_H, 3 * S2W], F32, tag="o3", name="o3", bufs=1)
            rv = rn[:, 0:S2W].copy()
            rv.ap = mybir.VecI64Pair([list(rv.ap[0]), [0, 3], [1, S2W]])
            TT(_chap(o3, S2W, 0, 1, 3, S2W), _chap(m3, S2W, 0, 1, 3, S2W), rv, OP.mult)
            for i in range(3):
                nc.sync.dma_start(
                    d_norm[i, t * TILE_H : (t + 1) * TILE_H, :],
                    o3[:, i * S2W : (i + 1) * S2W],
                )


def _host_constants(Kinv):
    """Per-core constant tensors derived from one [3,3] inverse intrinsic."""
    Ki = Kinv.astype(np.float32)
    band = np.zeros((128, TILE_H), np.float32)
    for k in range(128):
        lo, hi = max(0, k - 2 * HALO), min(TILE_H - 1, k)
        if lo <= hi:
            band[k, lo : hi + 1] = 1.0
    xg = np.tile((np.arange(PW, dtype=np.float32) - HALO)[None, :], (128, 1))
    ysc = np.zeros((128, 3 * N_TILES), np.float32)
    p = np.arange(128, dtype=np.float32)
    for t in range(N_TILES):
        hrow = t * TILE_H - HALO + p
        for i in range(3):
            ysc[:, 3 * t + i] = Ki[i, 1] * hrow + Ki[i, 2]
    kx = np.tile(Ki[:, 0][None, :], (128, 1)).astype(np.float32)
    return {"consts": np.concatenate([band, ysc, kx, xg], axis=1)}


_PROGRAM = None


def _program():
    global _PROGRAM
    if _PROGRAM is None:
        _PROGRAM = _build_program()
    return _PROGRAM


def _run(depth, intrinsic_inv, trace=False, **kw):
    nc = _program()
    in_maps = []
    for b in range(B):
        m = {"depth": np.ascontiguousarray(depth[b], dtype=np.float32)}
        m.update(_host_constants(np.asarray(intrinsic_inv[b])))
        in_maps.append(m)
    return run_bass_kernel_spmd(nc, in_maps, core_ids=list(range(B)), trace=trace, **kw)


def kernel(depth, intrinsic_inv):
    res = _run(np.asarray(depth), np.asarray(intrinsic_inv), trace=False)
    normals = np.stack([res.results[b]["normals"] for b in range(B)])
    points = np.stack([res.results[b]["points"] for b in range(B)])
    return normals.astype(np.float32), points.astype(np.float32)
